# revision 21
# baseline (speedup 1.0000x reference)
"""Causal self-attention (B=4, T=2048, C=768, H=12) on 8 TRN2 NeuronCores.

Sharding: (batch x head-half). Core c handles batch b = c//2 and heads
hg*6..hg*6+5 where hg = c%2. Each core computes qkv projection for its
1152 W_attn columns, causal attention for its 6 heads, and a partial
c_proj using its 384 rows of W_proj. Host sums the pair partials + bias.

Device-side layout trick: qkv is computed directly in transposed form
(qkvT = W.T @ x.T), so Q^T/K^T land in [64, T] per head (the layout the
S^T matmul wants) and V is computed in natural [T, 64] layout with an
extra ones-column, so the P^T @ V' matmul yields both O^T and the
softmax row-sums in one pass. The causal mask is applied by zeroing
exp(S^T) above the diagonal with gpsimd.affine_select. Per-query
normalization: reciprocal_approx_fast on the sums row, partition
broadcast, and a fused multiply during the PSUM->SBUF evacuation.
"""

import sys

import numpy as np

try:
    import concourse  # noqa: F401
except ImportError:
    sys.path.insert(0, "/opt/trn_rl_repo")

B, T, C, H, D = 4, 2048, 768, 12, 64
NH = H // 2          # heads per core
CH = NH * D          # 384 channels per core
NCB = C // 128       # 6 contraction blocks
NTB = T // 128       # 16 t-blocks
NQC = T // 512       # 4 query chunks
VW = D + 1           # 65: V plus ones column

_CACHE = {}


def _build_nc(probes=False, reps=1):
    from concourse import bacc, mybir, tile

    f32 = mybir.dt.float32
    f32r = mybir.dt.float32r
    bf16 = mybir.dt.bfloat16
    AF = mybir.ActivationFunctionType
    ALU = mybir.AluOpType

    nc = bacc.Bacc("TRN2", target_bir_lowering=False, debug=False, num_devices=8)
    if probes:
        dbg_qkT_d = nc.dram_tensor("dbg_qkT", [128, 6, T], bf16, kind="ExternalOutput")
        dbg_v1_d = nc.dram_tensor("dbg_v1", [128, NTB, NH * VW], f32, kind="ExternalOutput")
        dbg_pt_d = nc.dram_tensor("dbg_pt", [2, 128, 1024], f32, kind="ExternalOutput")
        dbg_rbb_d = nc.dram_tensor("dbg_rbb", [NH, 64, 512], f32, kind="ExternalOutput")
        dbg_opv_d = nc.dram_tensor("dbg_opv", [NH, VW, 512], f32, kind="ExternalOutput")
        dbg_ot_d = nc.dram_tensor("dbg_ot", [NH, 64, 512], f32, kind="ExternalOutput")

    xt_d = nc.dram_tensor("xt", [C, T], f32, kind="ExternalInput")
    wqk_d = nc.dram_tensor("wqk", [C, 2 * CH], f32, kind="ExternalInput")
    wv_d = nc.dram_tensor("wv", [C, CH], f32, kind="ExternalInput")
    bqk_d = nc.dram_tensor("bqk", [6, 128], f32, kind="ExternalInput")
    bv_d = nc.dram_tensor("bv", [1, CH], f32, kind="ExternalInput")
    wp_d = nc.dram_tensor("wp", [64, NH * C], f32, kind="ExternalInput")
    out_d = nc.dram_tensor("out", [T, C], f32, kind="ExternalOutput")

    with tile.TileContext(nc) as tc:
        with (
            tc.tile_pool(name="const", bufs=1) as cp,
            tc.tile_pool(name="work", bufs=2) as wk,
            tc.tile_pool(name="pt", bufs=3) as ptp,
            tc.tile_pool(name="ot", bufs=8) as otp,
            tc.tile_pool(name="outs", bufs=2) as osp,
            tc.tile_pool(name="ps_s", bufs=2, space="PSUM") as ps_s,
            tc.tile_pool(name="ps_o", bufs=2, space="PSUM") as ps_o,
            tc.tile_pool(name="ps_p", bufs=1, space="PSUM") as ps_p,
        ):
          for _rep in range(reps):
            # ---- resident inputs ----
            xt_sb = cp.tile([128, NCB, T], f32r, tag="xt")
            nc.sync.dma_start(out=xt_sb, in_=xt_d.rearrange("(n p) m -> p n m", p=128).bitcast(f32r))
            wqk_sb = cp.tile([128, NCB, 2 * CH], f32r, tag="wqk")
            nc.sync.dma_start(out=wqk_sb, in_=wqk_d.rearrange("(n p) m -> p n m", p=128).bitcast(f32r))
            wv_sb = cp.tile([128, NCB, CH], f32r, tag="wv")
            nc.sync.dma_start(out=wv_sb, in_=wv_d.rearrange("(n p) m -> p n m", p=128).bitcast(f32r))
            wp_sb = cp.tile([64, NH, C], f32r, tag="wp")
            nc.sync.dma_start(out=wp_sb, in_=wp_d.rearrange("p (n m) -> p n m", n=NH).bitcast(f32r))
            bqk_sb = cp.tile([128, 6], f32, tag="bqk")
            nc.sync.dma_start(out=bqk_sb, in_=bqk_d.rearrange("n p -> p n"))
            bv_sb = cp.tile([1, CH], f32, tag="bv")
            nc.sync.dma_start(out=bv_sb, in_=bv_d.ap())
            bvb_sb = cp.tile([128, CH], f32, tag="bvb")
            nc.gpsimd.partition_broadcast(bvb_sb, bv_sb)

            # ---- outputs of the qkv projection ----
            qkT = cp.tile([128, 6, T], bf16, tag="qkT")  # rows: Q blocks 0-2, K blocks 3-5
            v1 = cp.tile([128, NTB, NH * VW], f32r, tag="v1")
            v1_4d = v1.rearrange("p n (h e) -> p n h e", e=VW)
            nc.vector.memset(v1_4d[:, :, :, D].bitcast(f32), 1.0)  # ones columns

            # qkvT: Q^T and K^T, [c_out 128-block, t] = sum_c W[c, c_out] * xT[c, t]
            for co in range(6):
                for j in range(NQC):
                    ps = ps_s.tile([128, 1024], f32, tag="s")
                    for ci in range(NCB):
                        nc.tensor.matmul(
                            ps[:, 0:512],
                            lhsT=wqk_sb[:, ci, co * 128:(co + 1) * 128],
                            rhs=xt_sb[:, ci, j * 512:(j + 1) * 512],
                            start=(ci == 0),
                            stop=(ci == NCB - 1),
                        )
                    # evac with bias (+0.125 scale folded into Q); writes bf16
                    nc.scalar.activation(
                        out=qkT[:, co, j * 512:(j + 1) * 512],
                        in_=ps[:, 0:512],
                        func=AF.Identity,
                        bias=bqk_sb[:, co:co + 1],
                        scale=0.125 if co < 3 else 1.0,
                    )

            # V natural: [t 128-block, 384] = sum_c xT[c, t].T @ Wv[c, :]
            for tb in range(NTB):
                psv = ps_o.tile([128, 512], f32, tag="o")
                for ci in range(NCB):
                    nc.tensor.matmul(
                        psv[:, 0:CH],
                        lhsT=xt_sb[:, ci, tb * 128:(tb + 1) * 128],
                        rhs=wv_sb[:, ci, :],
                        start=(ci == 0),
                        stop=(ci == NCB - 1),
                    )
                psv_3d = psv[:, 0:CH].rearrange("p (h e) -> p h e", e=D)
                bvb_3d = bvb_sb.rearrange("p (h e) -> p h e", e=D)
                nc.vector.tensor_add(v1_4d[:, tb, :, 0:D], psv_3d, bvb_3d)

            # ---- attention + projection, per query chunk ----
            for j in range(NQC):
                ot_tiles = []
                for h in range(NH):
                    ht, hp = h // 2, (h % 2) * 64
                    nkb = 4 * (j + 1)  # causal 128-key-blocks
                    ps_pv = ps_o.tile([128, 512], f32, tag="o")
                    for pti in range(nkb // 2):
                        ps = ps_s.tile([128, 1024], f32, tag="s")
                        for half in range(2):
                            kb = pti * 2 + half
                            nc.tensor.matmul(
                                ps[:, half * 512:(half + 1) * 512],
                                lhsT=qkT[hp:hp + 64, 3 + ht, kb * 128:(kb + 1) * 128],
                                rhs=qkT[hp:hp + 64, ht, j * 512:(j + 1) * 512],
                                start=True,
                                stop=True,
                            )
                        pt = ptp.tile([128, 1024], f32r, tag="pt")
                        nc.scalar.activation(out=pt, in_=ps, func=AF.Exp)
                        _dump_pt = probes and j == 0 and h == 0
                        for half in range(2):
                            kb = pti * 2 + half
                            delta = kb * 128 - j * 512
                            if delta >= 0:  # diagonal block: zero q < k + delta
                                w = delta + 128
                                nc.gpsimd.affine_select(
                                    out=pt[:, half * 512:half * 512 + w],
                                    in_=pt[:, half * 512:half * 512 + w],
                                    compare_op=ALU.is_ge,
                                    fill=0.0,
                                    base=-delta,
                                    pattern=[[1, w]],
                                    channel_multiplier=-1,
                                )
                        if _dump_pt:
                            nc.sync.dma_start(
                                out=dbg_pt_d[pti], in_=pt.bitcast(f32)
                            )
                        for half in range(2):
                            kb = pti * 2 + half
                            nc.tensor.matmul(
                                ps_pv[0:VW, :],
                                lhsT=v1_4d[:, kb, h, :],
                                rhs=pt[:, half * 512:(half + 1) * 512],
                                start=(kb == 0),
                                stop=(kb == nkb - 1),
                            )
                    if probes and j == 0:
                        opv_sb = wk.tile([VW, 512], f32, tag="opv")
                        nc.scalar.copy(opv_sb, ps_pv[0:VW, :])
                        nc.sync.dma_start(out=dbg_opv_d[h], in_=opv_sb)
                    # normalize: O^T[d, q] / sums[q]. gpsimd/custom-DVE ops misread
                    # APs at base partition 64, so move the sums row to
                    # partition 0 first (ACT copy + SBUF->SBUF DMA shift).
                    srow = wk.tile([VW, 512], f32, tag="srow")
                    nc.scalar.copy(srow[D:VW, :], ps_pv[D:VW, :])
                    s0 = wk.tile([1, 512], f32, tag="s0")
                    nc.sync.dma_start(out=s0, in_=srow[D:VW, :])
                    r0 = wk.tile([1, 512], f32, tag="r0")
                    nc.vector.reciprocal_approx_fast(out=r0, in_=s0)
                    rbb = wk.tile([64, 512], f32, tag="rbb")
                    nc.gpsimd.partition_broadcast(rbb, r0)
                    ot = otp.tile([64, 512], f32r, tag="ot")
                    nc.vector.tensor_mul(ot, ps_pv[0:D, :], rbb)
                    ot_tiles.append(ot)
                    if probes and j == 0:
                        nc.sync.dma_start(out=dbg_rbb_d[h], in_=rbb)
                        nc.sync.dma_start(out=dbg_ot_d[h], in_=ot.bitcast(f32))

                # partial c_proj for this chunk's 4 t-blocks
                for tb4 in range(4):
                    tb = j * 4 + tb4
                    psp = ps_p.tile([128, 1024], f32, tag="p")
                    for lo, n in ((0, 512), (512, 256)):
                        for h in range(NH):
                            nc.tensor.matmul(
                                psp[:, lo:lo + n],
                                lhsT=ot_tiles[h][:, tb4 * 128:(tb4 + 1) * 128],
                                rhs=wp_sb[:, h, lo:lo + n],
                                start=(h == 0),
                                stop=(h == NH - 1),
                            )
                    ost = osp.tile([128, C], f32, tag="ost")
                    nc.vector.tensor_copy(ost, psp[:, 0:C])
                    nc.sync.dma_start(
                        out=out_d[tb * 128:(tb + 1) * 128, :], in_=ost
                    )

            if probes:
                nc.sync.dma_start(out=dbg_qkT_d.ap(), in_=qkT)
                nc.sync.dma_start(out=dbg_v1_d.ap(), in_=v1.bitcast(f32))

    nc.compile()
    return nc


def _shard_inputs(x, W_attn, b_attn, W_proj):
    in_maps = []
    for c in range(8):
        b, hg = c // 2, c % 2
        q0, k0, v0 = hg * CH, C + hg * CH, 2 * C + hg * CH
        bqk = np.concatenate(
            [b_attn[q0:q0 + CH] * 0.125, b_attn[k0:k0 + CH]]
        ).reshape(6, 128)
        in_maps.append({
            "xt": np.ascontiguousarray(x[b].T, dtype=np.float32),
            "wqk": np.ascontiguousarray(
                np.concatenate(
                    [W_attn[:, q0:q0 + CH], W_attn[:, k0:k0 + CH]], axis=1
                ),
                dtype=np.float32,
            ),
            "wv": np.ascontiguousarray(W_attn[:, v0:v0 + CH], dtype=np.float32),
            "bqk": np.ascontiguousarray(bqk, dtype=np.float32),
            "bv": np.ascontiguousarray(
                b_attn[v0:v0 + CH].reshape(1, CH), dtype=np.float32
            ),
            "wp": np.ascontiguousarray(
                W_proj[hg * CH:(hg + 1) * CH, :]
                .reshape(NH, 64, C)
                .transpose(1, 0, 2)
                .reshape(64, NH * C),
                dtype=np.float32,
            ),
        })
    return in_maps


def kernel(x, W_attn, b_attn, W_proj, b_proj, _trace=False):
    from concourse.bass_utils import run_bass_kernel_spmd

    x = np.asarray(x, dtype=np.float32)
    W_attn = np.asarray(W_attn, dtype=np.float32)
    b_attn = np.asarray(b_attn, dtype=np.float32)
    W_proj = np.asarray(W_proj, dtype=np.float32)
    b_proj = np.asarray(b_proj, dtype=np.float32)

    if "nc" not in _CACHE:
        _CACHE["nc"] = _build_nc()
    nc = _CACHE["nc"]

    in_maps = _shard_inputs(x, W_attn, b_attn, W_proj)
    res = run_bass_kernel_spmd(nc, in_maps, list(range(8)), trace=_trace)
    _CACHE["last_result"] = res

    out = np.empty((B, T, C), dtype=np.float32)
    for b in range(B):
        out[b] = res.results[2 * b]["out"] + res.results[2 * b + 1]["out"] + b_proj
    return out


# revision 24
# speedup vs baseline: 1.0761x; 1.0761x over previous
"""Causal self-attention (B=4, T=2048, C=768, H=12) on 8 TRN2 NeuronCores.

Sharding: (batch x head-half). Core c handles batch b = c//2 and heads
hg*6..hg*6+5 where hg = c%2. Each core computes qkv projection for its
1152 W_attn columns, causal attention for its 6 heads, and a partial
c_proj using its 384 rows of W_proj. Host sums the pair partials + bias.

Device-side layout trick: qkv is computed directly in transposed form
(qkvT = W.T @ x.T), so Q^T/K^T land in [64, T] per head (the layout the
S^T matmul wants) and V is computed in natural [T, 64] layout with an
extra ones-column, so the P^T @ V' matmul yields both O^T and the
softmax row-sums in one pass. The causal mask is applied by zeroing
exp(S^T) above the diagonal with gpsimd.affine_select. Per-query
normalization: reciprocal_approx_fast on the sums row, partition
broadcast, and a fused multiply during the PSUM->SBUF evacuation.
"""

import sys

import numpy as np

try:
    import concourse  # noqa: F401
except ImportError:
    sys.path.insert(0, "/opt/trn_rl_repo")

B, T, C, H, D = 4, 2048, 768, 12, 64
NH = H // 2          # heads per core
CH = NH * D          # 384 channels per core
NCB = C // 128       # 6 contraction blocks
NTB = T // 128       # 16 t-blocks
NQC = T // 512       # 4 query chunks
VW = D + 1           # 65: V plus ones column

_CACHE = {}


def _build_nc(probes=False, reps=1):
    from concourse import bacc, mybir, tile

    f32 = mybir.dt.float32
    f32r = mybir.dt.float32r
    bf16 = mybir.dt.bfloat16
    AF = mybir.ActivationFunctionType
    ALU = mybir.AluOpType

    nc = bacc.Bacc("TRN2", target_bir_lowering=False, debug=False, num_devices=8)
    if probes:
        dbg_qkT_d = nc.dram_tensor("dbg_qkT", [128, 6, T], bf16, kind="ExternalOutput")
        dbg_v1_d = nc.dram_tensor("dbg_v1", [128, NTB, NH * VW], f32, kind="ExternalOutput")
        dbg_pt_d = nc.dram_tensor("dbg_pt", [2, 128, 1024], f32, kind="ExternalOutput")
        dbg_rbb_d = nc.dram_tensor("dbg_rbb", [NH, 64, 512], f32, kind="ExternalOutput")
        dbg_opv_d = nc.dram_tensor("dbg_opv", [NH, VW, 512], f32, kind="ExternalOutput")
        dbg_ot_d = nc.dram_tensor("dbg_ot", [NH, 64, 512], f32, kind="ExternalOutput")

    xt_d = nc.dram_tensor("xt", [C, T], f32, kind="ExternalInput")
    wqk_d = nc.dram_tensor("wqk", [C, 2 * CH], f32, kind="ExternalInput")
    wv_d = nc.dram_tensor("wv", [C, CH], f32, kind="ExternalInput")
    bqk_d = nc.dram_tensor("bqk", [6, 128], f32, kind="ExternalInput")
    bv_d = nc.dram_tensor("bv", [1, CH], f32, kind="ExternalInput")
    wp_d = nc.dram_tensor("wp", [64, NH * C], f32, kind="ExternalInput")
    out_d = nc.dram_tensor("out", [T, C], f32, kind="ExternalOutput")

    with tile.TileContext(nc) as tc:
        with (
            tc.tile_pool(name="const", bufs=1) as cp,
            tc.tile_pool(name="work", bufs=2) as wk,
            tc.tile_pool(name="pt", bufs=4) as ptp,
            tc.tile_pool(name="ot", bufs=8) as otp,
            tc.tile_pool(name="outs", bufs=2) as osp,
            tc.tile_pool(name="ps_s", bufs=2, space="PSUM") as ps_s,
            tc.tile_pool(name="ps_o", bufs=2, space="PSUM") as ps_o,
            tc.tile_pool(name="ps_p", bufs=1, space="PSUM") as ps_p,
        ):
          for _rep in range(reps):
            # ---- resident inputs ----
            xt_sb = cp.tile([128, NCB, T], f32r, tag="xt")
            nc.sync.dma_start(out=xt_sb, in_=xt_d.rearrange("(n p) m -> p n m", p=128).bitcast(f32r))
            wqk_sb = cp.tile([128, NCB, 2 * CH], f32r, tag="wqk")
            nc.sync.dma_start(out=wqk_sb, in_=wqk_d.rearrange("(n p) m -> p n m", p=128).bitcast(f32r))
            wv_sb = cp.tile([128, NCB, CH], f32r, tag="wv")
            nc.sync.dma_start(out=wv_sb, in_=wv_d.rearrange("(n p) m -> p n m", p=128).bitcast(f32r))
            wp_sb = cp.tile([64, NH, C], f32r, tag="wp")
            nc.sync.dma_start(out=wp_sb, in_=wp_d.rearrange("p (n m) -> p n m", n=NH).bitcast(f32r))
            bqk_sb = cp.tile([128, 6], f32, tag="bqk")
            nc.sync.dma_start(out=bqk_sb, in_=bqk_d.rearrange("n p -> p n"))
            bv_sb = cp.tile([1, CH], f32, tag="bv")
            nc.sync.dma_start(out=bv_sb, in_=bv_d.ap())
            bvb_sb = cp.tile([128, CH], f32, tag="bvb")
            nc.gpsimd.partition_broadcast(bvb_sb, bv_sb)

            # ---- outputs of the qkv projection ----
            qkT = cp.tile([128, 6, T], bf16, tag="qkT")  # rows: Q blocks 0-2, K blocks 3-5
            v1 = cp.tile([128, NTB, NH * VW], f32r, tag="v1")
            v1_4d = v1.rearrange("p n (h e) -> p n h e", e=VW)
            nc.vector.memset(v1_4d[:, :, :, D].bitcast(f32), 1.0)  # ones columns

            # qkvT: Q^T and K^T, [c_out 128-block, t] = sum_c W[c, c_out] * xT[c, t]
            for co in range(6):
                for j in range(NQC):
                    ps = ps_s.tile([128, 1024], f32, tag="s")
                    for ci in range(NCB):
                        nc.tensor.matmul(
                            ps[:, 0:512],
                            lhsT=wqk_sb[:, ci, co * 128:(co + 1) * 128],
                            rhs=xt_sb[:, ci, j * 512:(j + 1) * 512],
                            start=(ci == 0),
                            stop=(ci == NCB - 1),
                        )
                    # evac with bias (+0.125 scale folded into Q); writes bf16
                    # (on DVE to keep ScalarE free for the exp stream)
                    nc.vector.tensor_scalar(
                        out=qkT[:, co, j * 512:(j + 1) * 512],
                        in0=ps[:, 0:512],
                        scalar1=0.125 if co < 3 else 1.0,
                        scalar2=bqk_sb[:, co:co + 1],
                        op0=ALU.mult,
                        op1=ALU.add,
                    )

            # V natural: [t 128-block, 384] = sum_c xT[c, t].T @ Wv[c, :]
            for tb in range(NTB):
                psv = ps_o.tile([128, 512], f32, tag="o")
                for ci in range(NCB):
                    nc.tensor.matmul(
                        psv[:, 0:CH],
                        lhsT=xt_sb[:, ci, tb * 128:(tb + 1) * 128],
                        rhs=wv_sb[:, ci, :],
                        start=(ci == 0),
                        stop=(ci == NCB - 1),
                    )
                psv_3d = psv[:, 0:CH].rearrange("p (h e) -> p h e", e=D)
                bvb_3d = bvb_sb.rearrange("p (h e) -> p h e", e=D)
                nc.vector.tensor_add(v1_4d[:, tb, :, 0:D], psv_3d, bvb_3d)

            # ---- attention + projection, per query chunk ----
            for j in range(NQC):
                ot_tiles = []
                for h in range(NH):
                    ht, hp = h // 2, (h % 2) * 64
                    nkb = 4 * (j + 1)  # causal 128-key-blocks
                    ps_pv = ps_o.tile([128, 512], f32, tag="o")
                    for pti in range(nkb // 2):
                        ps = ps_s.tile([128, 1024], f32, tag="s")
                        for half in range(2):
                            kb = pti * 2 + half
                            nc.tensor.matmul(
                                ps[:, half * 512:(half + 1) * 512],
                                lhsT=qkT[hp:hp + 64, 3 + ht, kb * 128:(kb + 1) * 128],
                                rhs=qkT[hp:hp + 64, ht, j * 512:(j + 1) * 512],
                                start=True,
                                stop=True,
                            )
                        pt = ptp.tile([128, 1024], f32r, tag="pt")
                        nc.scalar.activation(out=pt, in_=ps, func=AF.Exp)
                        _dump_pt = probes and j == 0 and h == 0
                        for half in range(2):
                            kb = pti * 2 + half
                            delta = kb * 128 - j * 512
                            if delta >= 0:  # diagonal block: zero q < k + delta
                                w = delta + 128
                                nc.gpsimd.affine_select(
                                    out=pt[:, half * 512:half * 512 + w],
                                    in_=pt[:, half * 512:half * 512 + w],
                                    compare_op=ALU.is_ge,
                                    fill=0.0,
                                    base=-delta,
                                    pattern=[[1, w]],
                                    channel_multiplier=-1,
                                )
                        if _dump_pt:
                            nc.sync.dma_start(
                                out=dbg_pt_d[pti], in_=pt.bitcast(f32)
                            )
                        for half in range(2):
                            kb = pti * 2 + half
                            nc.tensor.matmul(
                                ps_pv[0:VW, :],
                                lhsT=v1_4d[:, kb, h, :],
                                rhs=pt[:, half * 512:(half + 1) * 512],
                                start=(kb == 0),
                                stop=(kb == nkb - 1),
                            )
                    if probes and j == 0:
                        opv_sb = wk.tile([VW, 512], f32, tag="opv")
                        nc.scalar.copy(opv_sb, ps_pv[0:VW, :])
                        nc.sync.dma_start(out=dbg_opv_d[h], in_=opv_sb)
                    # normalize: O^T[d, q] / sums[q]. gpsimd/custom-DVE ops misread
                    # APs at base partition 64, so move the sums row to
                    # partition 0 first (ACT copy + SBUF->SBUF DMA shift).
                    srow = wk.tile([VW, 512], f32, tag="srow")
                    nc.vector.tensor_copy(srow[D:VW, :], ps_pv[D:VW, :])
                    s0 = wk.tile([1, 512], f32, tag="s0")
                    nc.sync.dma_start(out=s0, in_=srow[D:VW, :])
                    r0 = wk.tile([1, 512], f32, tag="r0")
                    nc.vector.reciprocal_approx_fast(out=r0, in_=s0)
                    rbb = wk.tile([64, 512], f32, tag="rbb")
                    nc.gpsimd.partition_broadcast(rbb, r0)
                    ot = otp.tile([64, 512], f32r, tag="ot")
                    nc.vector.tensor_mul(ot, ps_pv[0:D, :], rbb)
                    ot_tiles.append(ot)
                    if probes and j == 0:
                        nc.sync.dma_start(out=dbg_rbb_d[h], in_=rbb)
                        nc.sync.dma_start(out=dbg_ot_d[h], in_=ot.bitcast(f32))

                # partial c_proj for this chunk's 4 t-blocks
                for tb4 in range(4):
                    tb = j * 4 + tb4
                    psp = ps_p.tile([128, 1024], f32, tag="p")
                    for lo, n in ((0, 512), (512, 256)):
                        for h in range(NH):
                            nc.tensor.matmul(
                                psp[:, lo:lo + n],
                                lhsT=ot_tiles[h][:, tb4 * 128:(tb4 + 1) * 128],
                                rhs=wp_sb[:, h, lo:lo + n],
                                start=(h == 0),
                                stop=(h == NH - 1),
                            )
                    ost = osp.tile([128, C], f32, tag="ost")
                    nc.vector.tensor_copy(ost, psp[:, 0:C])
                    nc.sync.dma_start(
                        out=out_d[tb * 128:(tb + 1) * 128, :], in_=ost
                    )

            if probes:
                nc.sync.dma_start(out=dbg_qkT_d.ap(), in_=qkT)
                nc.sync.dma_start(out=dbg_v1_d.ap(), in_=v1.bitcast(f32))

    nc.compile()
    return nc


def _shard_inputs(x, W_attn, b_attn, W_proj):
    in_maps = []
    for c in range(8):
        b, hg = c // 2, c % 2
        q0, k0, v0 = hg * CH, C + hg * CH, 2 * C + hg * CH
        bqk = np.concatenate(
            [b_attn[q0:q0 + CH] * 0.125, b_attn[k0:k0 + CH]]
        ).reshape(6, 128)
        in_maps.append({
            "xt": np.ascontiguousarray(x[b].T, dtype=np.float32),
            "wqk": np.ascontiguousarray(
                np.concatenate(
                    [W_attn[:, q0:q0 + CH], W_attn[:, k0:k0 + CH]], axis=1
                ),
                dtype=np.float32,
            ),
            "wv": np.ascontiguousarray(W_attn[:, v0:v0 + CH], dtype=np.float32),
            "bqk": np.ascontiguousarray(bqk, dtype=np.float32),
            "bv": np.ascontiguousarray(
                b_attn[v0:v0 + CH].reshape(1, CH), dtype=np.float32
            ),
            "wp": np.ascontiguousarray(
                W_proj[hg * CH:(hg + 1) * CH, :]
                .reshape(NH, 64, C)
                .transpose(1, 0, 2)
                .reshape(64, NH * C),
                dtype=np.float32,
            ),
        })
    return in_maps


def kernel(x, W_attn, b_attn, W_proj, b_proj, _trace=False):
    from concourse.bass_utils import run_bass_kernel_spmd

    x = np.asarray(x, dtype=np.float32)
    W_attn = np.asarray(W_attn, dtype=np.float32)
    b_attn = np.asarray(b_attn, dtype=np.float32)
    W_proj = np.asarray(W_proj, dtype=np.float32)
    b_proj = np.asarray(b_proj, dtype=np.float32)

    if "nc" not in _CACHE:
        _CACHE["nc"] = _build_nc()
    nc = _CACHE["nc"]

    in_maps = _shard_inputs(x, W_attn, b_attn, W_proj)
    res = run_bass_kernel_spmd(nc, in_maps, list(range(8)), trace=_trace)
    _CACHE["last_result"] = res

    out = np.empty((B, T, C), dtype=np.float32)
    for b in range(B):
        out[b] = res.results[2 * b]["out"] + res.results[2 * b + 1]["out"] + b_proj
    return out
